# revision 1
# baseline (speedup 1.0000x reference)
"""Trainium2 Bass kernel for nn_MultiHeadAttention_83863531421896.

Full-input contract: kernel(**inputs) takes the unsharded tensors and
returns the full (2, 2048, 1024) output. Internally the 16 heads are
sharded 2-per-core across 8 NeuronCores (tensor parallel); each core
computes its heads' attention plus its slice of the output projection,
and the 8 partial projections are reduced on the host.

Device dataflow per core (heads h0, h1):
  per batch b:
    qkvT = W_qkv_slice @ x^T  (fp32r, transposed layout [q_h0;q_h1],
           [k_h0;k_h1] stacked 64+64 partitions, vT), V^T -> V via PE
           transposes, V packed as [V | ones] blocks
    attention: per (q-chunk, kk-pair): S^T = K Q^T on dual 64-row PE
           tiles (both heads concurrently), exp((1/8) S^T) on ScalarE
           from PSUM, A^T V via [V|ones] stationary operand ->
           attention output + softmax denominators in one accumulation;
           normalization deferred to eviction (approx reciprocal)
    out-proj for the previous batch interleaves with the next batch's
           qkv so its DMA hides under attention
"""

import sys

if "/opt/trn_rl_repo" not in sys.path:
    sys.path.insert(0, "/opt/trn_rl_repo")

import numpy as np

B = 2
S = 2048
D = 1024
H = 16
HD = 64
N_CORES = 8
HEADS_PER_CORE = H // N_CORES  # 2
M = B * S                      # 4096 tokens
N_MCHUNK_B = S // 512          # 4 m-chunks of 512 tokens per batch
N_KTILE = D // 128             # 8 contraction tiles for qkv
N_QCHUNK = S // 512            # 4 q-chunks per batch
N_KKTILE = S // 128            # 16 key tiles per batch
SCALE = 1.0 / np.sqrt(HD)

_CACHE = {}


def _build_module():
    import concourse.bass as bass
    import concourse.tile as tile
    from concourse import bacc, mybir

    f32 = mybir.dt.float32
    f32r = mybir.dt.float32r
    Exp = mybir.ActivationFunctionType.Exp
    Copy = mybir.ActivationFunctionType.Copy
    Ident = mybir.ActivationFunctionType.Identity

    nc = bacc.Bacc("TRN2", target_bir_lowering=False, debug=False,
                   num_devices=N_CORES)

    xt_ap = nc.dram_tensor("xt", [D, M], f32r, kind="ExternalInput").ap()
    wqa_ap = nc.dram_tensor("wqa", [D, 128], f32r, kind="ExternalInput").ap()
    wqb_ap = nc.dram_tensor("wqb", [D, 128], f32r, kind="ExternalInput").ap()
    wv_ap = nc.dram_tensor("wv", [D, 128], f32r, kind="ExternalInput").ap()
    wo_ap = nc.dram_tensor("wo", [128, D], f32r, kind="ExternalInput").ap()
    ba_ap = nc.dram_tensor("ba", [128, 1], f32, kind="ExternalInput").ap()
    bb_ap = nc.dram_tensor("bb", [128, 1], f32, kind="ExternalInput").ap()
    bv_ap = nc.dram_tensor("bv", [128, 1], f32, kind="ExternalInput").ap()
    ones_ap = nc.dram_tensor("ones", [128, 64], f32r, kind="ExternalInput").ap()
    ident_ap = nc.dram_tensor("ident", [128, 128], f32r, kind="ExternalInput").ap()
    out_ap = nc.dram_tensor("partial", [D, M], f32, kind="ExternalOutput").ap()
    sums_dram = nc.dram_tensor(
        "sums_scratch", [B * N_QCHUNK * HEADS_PER_CORE, 512], f32).ap()

    with tile.TileContext(nc) as tc:
        with tc.tile_pool(name="persist", bufs=1) as persist, \
             tc.tile_pool(name="const", bufs=1) as const, \
             tc.tile_pool(name="xpool", bufs=4) as xpool, \
             tc.tile_pool(name="vt_pool", bufs=2) as vt_pool, \
             tc.tile_pool(name="ps8", bufs=1, space="PSUM") as ps8, \
             tc.tile_pool(name="epool", bufs=2) as epool, \
             tc.tile_pool(name="stage", bufs=2) as stage, \
             tc.tile_pool(name="fin", bufs=4) as fin:
            qka_sb = persist.tile([128, M], f32r, tag="qka")
            qkb_sb = persist.tile([128, M], f32r, tag="qkb")
            v_sb = persist.tile([128, B, N_KKTILE, HEADS_PER_CORE, 65], f32r,
                                tag="vsb")
            outt_sb = persist.tile([128, M], f32r, tag="outt")

            wo_sb = const.tile([128, D], f32r, tag="wo")
            nc.scalar.dma_start(wo_sb[:], wo_ap[:])
            ident_sb = const.tile([128, 128], f32r, tag="ident")
            nc.scalar.dma_start(ident_sb[:], ident_ap[:])
            ba_sb = const.tile([128, 1], f32, tag="ba")
            nc.scalar.dma_start(ba_sb[:], ba_ap[:])
            bb_sb = const.tile([128, 1], f32, tag="bb")
            nc.scalar.dma_start(bb_sb[:], bb_ap[:])
            bv_sb = const.tile([128, 1], f32, tag="bv")
            nc.scalar.dma_start(bv_sb[:], bv_ap[:])
            wq_sb = const.tile([128, 3, N_KTILE, 128], f32r, tag="wq")
            for ki in range(N_KTILE):
                eng = (nc.gpsimd, nc.scalar, nc.gpsimd)[ki % 3]
                eng.dma_start(wq_sb[:, 0, ki], wqa_ap[ki * 128:(ki + 1) * 128, :])
                eng.dma_start(wq_sb[:, 1, ki], wqb_ap[ki * 128:(ki + 1) * 128, :])
                eng.dma_start(wq_sb[:, 2, ki], wv_ap[ki * 128:(ki + 1) * 128, :])
            nc.gpsimd.dma_start(
                v_sb[:, :, :, :, 64:65],
                ones_ap[:, 0:B * N_KKTILE * HEADS_PER_CORE].rearrange(
                    "p (b t h) -> p b t h", b=B, t=N_KKTILE)[:, :, :, :, None])

            def qkv_phase(b2):
                vt_sb = vt_pool.tile([128, S], f32r, tag="vt", name=f"vt{b2}")
                xss = []
                for mc in range(N_MCHUNK_B):
                    mi = b2 * N_MCHUNK_B + mc
                    xs = xpool.tile([128, N_KTILE, 512], f32r, tag="xs",
                                    name=f"xs{mi}")
                    for ki in range(N_KTILE):
                        eng = (nc.sync, nc.gpsimd, nc.sync, nc.scalar,
                               nc.sync, nc.gpsimd, nc.sync, nc.scalar)[ki]
                        eng.dma_start(
                            xs[:, ki],
                            xt_ap[ki * 128:(ki + 1) * 128, mi * 512:(mi + 1) * 512])
                    xss.append(xs)
                # ki-inner-most over m-chunks: one weight load feeds 4 matmuls
                for ei, (bias, dest) in enumerate(
                        [(ba_sb, qka_sb), (bb_sb, qkb_sb), (bv_sb, vt_sb)]):
                    pss = [ps8.tile([128, 512], f32, tag=f"av{mc // 2}{mc % 2}",
                                    name=f"qkvps{mc}") for mc in range(N_MCHUNK_B)]
                    for ki in range(N_KTILE):
                        for mc in range(N_MCHUNK_B):
                            nc.tensor.matmul(pss[mc][:], wq_sb[:, ei, ki],
                                             xss[mc][:, ki],
                                             start=(ki == 0), stop=(ki == N_KTILE - 1))
                    for mc in range(N_MCHUNK_B):
                        col = (b2 * N_MCHUNK_B + mc) if ei < 2 else mc
                        nc.vector.tensor_scalar_add(
                            dest[:, col * 512:(col + 1) * 512], pss[mc][:], bias[:])
                for kt in range(N_KKTILE):
                    tp = ps8.tile([128, 128], f32r, tag=f"av{kt % 2}1", name="tp")
                    nc.tensor.transpose(tp[:], vt_sb[:, kt * 128:(kt + 1) * 128],
                                        ident_sb[:])
                    for h in range(HEADS_PER_CORE):
                        nc.vector.tensor_copy(v_sb[:, b2, kt, h, 0:64],
                                              tp[:, h * 64:(h + 1) * 64])

            def attn_phase(b2):
                for qi in range(N_QCHUNK):
                    qcol = b2 * S + qi * 512
                    avp = [[ps8.tile([128, 512], f32, tag=f"av{h}{par}",
                                     name=f"av{h}{par}")
                            for par in range(2)] for h in range(HEADS_PER_CORE)]
                    def emit_av(kt, es_kt):
                        first = (kt == 0)
                        last = (kt == N_KKTILE - 1)
                        for h in range(HEADS_PER_CORE):
                            for par in range(2):
                                nc.tensor.matmul(
                                    avp[h][par][0:65, :],
                                    v_sb[par * 64:par * 64 + 64, b2, kt, h, :],
                                    es_kt[h][par * 64:par * 64 + 64, :],
                                    start=first, stop=last)

                    pending = None
                    for kt in range(N_KKTILE):
                        kkcol = b2 * S + kt * 128
                        scs = []
                        for h in range(HEADS_PER_CORE):
                            # scores: T0/T8 alternation (h0 parts 0:64, h1
                            # parts 64:128); bufs=2 so the next tile's scores
                            # don't wait on this tile's exp
                            sc = ps8.tile([128, 512], f32, tag=f"sc{h}",
                                          bufs=2, name=f"sc{h}")
                            nc.tensor.matmul(
                                sc[:],
                                qkb_sb[h * 64:(h + 1) * 64, kkcol:kkcol + 128],
                                qka_sb[h * 64:(h + 1) * 64, qcol:qcol + 512],
                                start=True, stop=True)
                            scs.append(sc)
                        es = []
                        for h in range(HEADS_PER_CORE):
                            e = epool.tile([128, 512], f32r, tag=f"e{h}",
                                           bufs=4, name=f"e{h}")
                            nc.scalar.activation(e[:], scs[h][:], Exp, scale=SCALE)
                            es.append(e)
                        if pending is not None:
                            emit_av(*pending)
                        pending = (kt, es)
                    emit_av(*pending)
                    for h in range(HEADS_PER_CORE):
                        st1 = stage.tile([128, 512], f32, tag="st1", name="st1")
                        nc.vector.tensor_copy(st1[0:65, :], avp[h][1][0:65, :])
                        st = stage.tile([128, 512], f32, tag="st", name="st")
                        nc.vector.tensor_add(st[0:65, :], avp[h][0][0:65, :],
                                             st1[0:65, :])
                        sidx = (b2 * N_QCHUNK + qi) * HEADS_PER_CORE + h
                        nc.gpsimd.dma_start(sums_dram[sidx:sidx + 1, :],
                                            st[64:65, :])
                        rb = stage.tile([128, 512], f32, tag="rb", name="rb")
                        nc.gpsimd.dma_start(
                            rb[0:64, :],
                            sums_dram[sidx:sidx + 1, :]
                            .partition_broadcast(64).squeeze(1))
                        rb2 = stage.tile([128, 512], f32, tag="rb2", name="rb2")
                        nc.vector.reciprocal_approx_fast(rb2[0:64, :], rb[0:64, :])
                        if h == 0:
                            nc.vector.tensor_mul(outt_sb[0:64, qcol:qcol + 512],
                                                 st[0:64, :], rb2[0:64, :])
                        else:
                            tm = stage.tile([128, 512], f32r, tag="tm", name="tm")
                            nc.vector.tensor_mul(tm[0:64, :], st[0:64, :],
                                                 rb2[0:64, :])
                            nc.gpsimd.dma_start(outt_sb[64:128, qcol:qcol + 512],
                                                tm[0:64, :])

            def outproj_chunk(b2, mc):
                # finalT[e, m-chunk] = wo_sb[:, e-tile].T @ outT[:, m-chunk]
                for et in range(D // 128):
                    mrow = b2 * S + mc * 512
                    fp = ps8.tile([128, 512], f32, tag=f"av{et % 2}0",
                                  name="fp")
                    nc.tensor.matmul(fp[:],
                                     wo_sb[:, et * 128:(et + 1) * 128],
                                     outt_sb[:, mrow:mrow + 512],
                                     start=True, stop=True)
                    fo = fin.tile([128, 512], f32, tag="fo", name="fo")
                    nc.vector.tensor_copy(fo[:], fp[:])
                    nc.sync.dma_start(
                        out_ap[et * 128:(et + 1) * 128, mrow:mrow + 512],
                        fo[:])

            qkv_phase(0)
            attn_phase(0)
            qkv_phase(1)
            for mc in range(N_MCHUNK_B):
                outproj_chunk(0, mc)
            tc.no_sync_barrier()
            attn_phase(1)
            tc.no_sync_barrier()
            for mc in range(N_MCHUNK_B):
                outproj_chunk(1, mc)
    nc.compile()
    return nc


def _shard_inputs(x, w_qkv, b_qkv, w_out):
    xt = np.ascontiguousarray(x.reshape(M, D).T)  # (1024, 4096)
    ones = np.ones((128, 64), dtype=np.float32)
    ident = np.eye(128, dtype=np.float32)
    in_maps = []
    for c in range(N_CORES):
        h0 = HEADS_PER_CORE * c
        rows_q, rows_k, rows_v, dcols = [], [], [], []
        for h in (h0, h0 + 1):
            rows_q += list(range(h * 192, h * 192 + 64))
            rows_k += list(range(h * 192 + 64, h * 192 + 128))
            rows_v += list(range(h * 192 + 128, h * 192 + 192))
            dcols += list(range(h * 64, (h + 1) * 64))
        in_maps.append({
            "xt": xt,
            "wqa": np.ascontiguousarray(w_qkv[rows_q, :].T),
            "wqb": np.ascontiguousarray(w_qkv[rows_k, :].T),
            "wv": np.ascontiguousarray(w_qkv[rows_v, :].T),
            "wo": np.ascontiguousarray(w_out[:, dcols].T),
            "ba": np.ascontiguousarray(b_qkv[rows_q].reshape(128, 1)),
            "bb": np.ascontiguousarray(b_qkv[rows_k].reshape(128, 1)),
            "bv": np.ascontiguousarray(b_qkv[rows_v].reshape(128, 1)),
            "ones": ones,
            "ident": ident,
        })
    return in_maps


def kernel(x, w_qkv, b_qkv, w_out, b_out, _trace=False):
    from concourse.bass_utils import run_bass_kernel_spmd

    x = np.asarray(x, dtype=np.float32)
    w_qkv = np.asarray(w_qkv, dtype=np.float32)
    b_qkv = np.asarray(b_qkv, dtype=np.float32)
    w_out = np.asarray(w_out, dtype=np.float32)
    b_out = np.asarray(b_out, dtype=np.float32)

    if "nc" not in _CACHE:
        _CACHE["nc"] = _build_module()
    nc = _CACHE["nc"]

    in_maps = _shard_inputs(x, w_qkv, b_qkv, w_out)
    res = run_bass_kernel_spmd(nc, in_maps, list(range(N_CORES)), trace=_trace)
    acc = np.zeros((D, M), dtype=np.float64)
    for c in range(N_CORES):
        acc += res.results[c]["partial"]
    acc = acc.T + b_out
    out = acc.astype(np.float32).reshape(B, S, D)
    if _trace:
        _CACHE["last_exec_time_ns"] = res.exec_time_ns
        _CACHE["last_res"] = res
    return out



# revision 9
# speedup vs baseline: 1.2617x; 1.2617x over previous
"""Trainium2 Bass kernel for nn_MultiHeadAttention_83863531421896.

Full-input contract: kernel(**inputs) takes the unsharded tensors and
returns the full (2, 2048, 1024) output. Internally the 16 heads are
sharded 2-per-core across 8 NeuronCores (tensor parallel); each core
computes its heads' attention plus its slice of the output projection,
and the 8 partial projections are reduced on the host.

v2 design notes (vs the fp32r baseline):
  - All matmul operands are bf16 (1 cycle/row on HW vs 2 for fp32),
    accumulation stays fp32 in PSUM. Host pre-casts x/weights; on-device
    casts are folded into existing PSUM->SBUF evictions for free.
  - AV uses the full 128-key contraction in one matmul (the fp32 version
    needed dual 64-row co-executing matmuls to reach the same rate).
  - exp runs on the Scalar engine in [128, 1024] tiles (both heads of a
    key-tile in one activation) to amortize access latency; the scalar
    queue carries nothing else. Scalar exp (~140us) and Tensor (~145us)
    are the co-bottlenecks and fully overlap.
  - Cross-phase software pipelining by emission order: qkv(b1) matmuls
    are injected into attn(b0)'s tensor-queue slots, outproj(b0) and
    most of outproj(b1) into attn(b1)'s slots, so the Tensor engine
    never idles while the Scalar engine streams exp.
"""

import sys

if "/opt/trn_rl_repo" not in sys.path:
    sys.path.insert(0, "/opt/trn_rl_repo")

import numpy as np

B = 2
S = 2048
D = 1024
H = 16
HD = 64
N_CORES = 8
HEADS_PER_CORE = H // N_CORES  # 2
M = B * S                      # 4096 tokens
N_MCHUNK_B = S // 512          # 4 m-chunks of 512 tokens per batch
N_KTILE = D // 128             # 8 contraction tiles for qkv
N_QCHUNK = S // 512            # 4 q-chunks per batch
N_KKTILE = S // 128            # 16 key tiles per batch
SCALE = 1.0 / np.sqrt(HD)

_CACHE = {}


def _build_module():
    import concourse.bass as bass
    import concourse.tile as tile
    from concourse import bacc, mybir

    f32 = mybir.dt.float32
    bf16 = mybir.dt.bfloat16
    Exp = mybir.ActivationFunctionType.Exp

    nc = bacc.Bacc("TRN2", target_bir_lowering=False, debug=False,
                   num_devices=N_CORES)

    xt_ap = nc.dram_tensor("xt", [D, M], bf16, kind="ExternalInput").ap()
    wqa_ap = nc.dram_tensor("wqa", [D, 128], bf16, kind="ExternalInput").ap()
    wqb_ap = nc.dram_tensor("wqb", [D, 128], bf16, kind="ExternalInput").ap()
    wv_ap = nc.dram_tensor("wv", [D, 128], bf16, kind="ExternalInput").ap()
    wo_ap = nc.dram_tensor("wo", [128, D], bf16, kind="ExternalInput").ap()
    ba_ap = nc.dram_tensor("ba", [128, 1], f32, kind="ExternalInput").ap()
    bb_ap = nc.dram_tensor("bb", [128, 1], f32, kind="ExternalInput").ap()
    bv_ap = nc.dram_tensor("bv", [128, 1], f32, kind="ExternalInput").ap()
    ones_ap = nc.dram_tensor("ones", [128, 64], bf16, kind="ExternalInput").ap()
    ident_ap = nc.dram_tensor("ident", [128, 128], bf16,
                              kind="ExternalInput").ap()
    out_ap = nc.dram_tensor("partial", [D, M], f32, kind="ExternalOutput").ap()
    sums_dram = nc.dram_tensor(
        "sums_scratch", [B * N_QCHUNK * HEADS_PER_CORE, 512], f32).ap()

    with tile.TileContext(nc) as tc:
        with tc.tile_pool(name="persist", bufs=1) as persist, \
             tc.tile_pool(name="const", bufs=1) as const, \
             tc.tile_pool(name="xpool", bufs=1) as xpool, \
             tc.tile_pool(name="vt_pool", bufs=2) as vt_pool, \
             tc.tile_pool(name="ps8", bufs=1, space="PSUM") as ps8, \
             tc.tile_pool(name="epool", bufs=1) as epool, \
             tc.tile_pool(name="stage", bufs=2) as stage, \
             tc.tile_pool(name="fin", bufs=4) as fin:
            qka_sb = persist.tile([128, M], bf16, tag="qka")
            qkb_sb = persist.tile([128, M], bf16, tag="qkb")
            v_sb = persist.tile([128, B, N_KKTILE, HEADS_PER_CORE, 65], bf16,
                                tag="vsb")
            outt_sb = persist.tile([128, M], bf16, tag="outt")

            wo_sb = const.tile([128, D], bf16, tag="wo")
            nc.gpsimd.dma_start(wo_sb[:], wo_ap[:])
            ident_sb = const.tile([128, 128], bf16, tag="ident")
            nc.gpsimd.dma_start(ident_sb[:], ident_ap[:])
            ba_sb = const.tile([128, 1], f32, tag="ba")
            nc.gpsimd.dma_start(ba_sb[:], ba_ap[:])
            bb_sb = const.tile([128, 1], f32, tag="bb")
            nc.gpsimd.dma_start(bb_sb[:], bb_ap[:])
            bv_sb = const.tile([128, 1], f32, tag="bv")
            nc.gpsimd.dma_start(bv_sb[:], bv_ap[:])
            wq_sb = const.tile([128, 3, N_KTILE, 128], bf16, tag="wq")
            for ki in range(N_KTILE):
                eng = (nc.gpsimd, nc.sync)[ki % 2]
                eng.dma_start(wq_sb[:, 0, ki], wqa_ap[ki * 128:(ki + 1) * 128, :])
                eng.dma_start(wq_sb[:, 1, ki], wqb_ap[ki * 128:(ki + 1) * 128, :])
                eng.dma_start(wq_sb[:, 2, ki], wv_ap[ki * 128:(ki + 1) * 128, :])
            nc.gpsimd.dma_start(
                v_sb[:, :, :, :, 64:65],
                ones_ap[:, 0:B * N_KKTILE * HEADS_PER_CORE].rearrange(
                    "p (b t h) -> p b t h", b=B, t=N_KKTILE)[:, :, :, :, None])

            # all of x staged up front; DMAs alternate sync/gpsimd queues
            xs = xpool.tile([128, B * N_MCHUNK_B, N_KTILE, 512], bf16, tag="xs")
            for mi in range(B * N_MCHUNK_B):
                for ki in range(N_KTILE):
                    eng = (nc.sync, nc.gpsimd)[(mi * N_KTILE + ki) % 2]
                    eng.dma_start(
                        xs[:, mi, ki],
                        xt_ap[ki * 128:(ki + 1) * 128, mi * 512:(mi + 1) * 512])

            vts = [None, None]

            def qkv_batch0():
                # before attention exists: use sc/av psum tags as 4 qkv banks
                vt_sb = vt_pool.tile([128, S], bf16, tag="vt", name="vt0")
                vts[0] = vt_sb
                for ei, (bias, dest) in enumerate(
                        [(ba_sb, qka_sb), (bb_sb, qkb_sb), (bv_sb, vt_sb)]):
                    pss = []
                    for mc, t in enumerate(("sc", "sc", "av0", "av1")):
                        kw = {"bufs": 2} if t == "sc" else {}
                        pss.append(ps8.tile([128, 512], f32, tag=t,
                                            name=f"qkvps{mc}", **kw))
                    for ki in range(N_KTILE):
                        for mc in range(N_MCHUNK_B):
                            nc.tensor.matmul(pss[mc][:], wq_sb[:, ei, ki],
                                             xs[:, mc, ki],
                                             start=(ki == 0),
                                             stop=(ki == N_KTILE - 1))
                    for mc in range(N_MCHUNK_B):
                        nc.vector.tensor_scalar_add(
                            dest[:, mc * 512:(mc + 1) * 512], pss[mc][:],
                            bias[:])
                for kt in range(N_KKTILE):
                    tp = ps8.tile([128, 128], bf16, tag="mm", bufs=2,
                                  name="tp0")
                    nc.tensor.transpose(tp[:], vt_sb[:, kt * 128:(kt + 1) * 128],
                                        ident_sb[:])
                    for h in range(HEADS_PER_CORE):
                        nc.vector.tensor_copy(v_sb[:, 0, kt, h, 0:64],
                                              tp[:, h * 64:(h + 1) * 64])

            def qkv_batch1_items():
                """Yield closures, each emitting ~one PE instruction of the
                b1 qkv phase, for injection into attn(b0)'s tensor stream."""
                b2 = 1
                vt_sb = vt_pool.tile([128, S], bf16, tag="vt", name="vt1")
                vts[1] = vt_sb
                # v first so transposes can spread out, then q, then k
                for ei, (bias, dest, dcol) in enumerate(
                        [(bv_sb, vt_sb, 0), (ba_sb, qka_sb, S),
                         (bb_sb, qkb_sb, S)]):
                    ei_w = (2, 0, 1)[ei]
                    for mc in range(N_MCHUNK_B):
                        mi = N_MCHUNK_B + mc
                        ps = ps8.tile([128, 512], f32, tag="mm", bufs=2,
                                      name=f"qkv1ps{ei}{mc}")
                        for ki in range(N_KTILE):
                            def mm(ki=ki, ps=ps, ei_w=ei_w, mi=mi):
                                nc.tensor.matmul(ps[:], wq_sb[:, ei_w, ki],
                                                 xs[:, mi, ki],
                                                 start=(ki == 0),
                                                 stop=(ki == N_KTILE - 1))
                            yield mm
                        def evict(ps=ps, dest=dest, bias=bias, dcol=dcol,
                                  mc=mc):
                            nc.vector.tensor_scalar_add(
                                dest[:, dcol + mc * 512:dcol + (mc + 1) * 512],
                                ps[:], bias[:])
                        yield evict
                        if ei == 0:
                            for kt in range(mc * 4, (mc + 1) * 4):
                                def tpf(kt=kt, vt_sb=vt_sb):
                                    tp = ps8.tile([128, 128], bf16, tag="mm",
                                                  bufs=2, name="tp1")
                                    nc.tensor.transpose(
                                        tp[:], vt_sb[:, kt * 128:(kt + 1) * 128],
                                        ident_sb[:])
                                    for h in range(HEADS_PER_CORE):
                                        nc.vector.tensor_copy(
                                            v_sb[:, 1, kt, h, 0:64],
                                            tp[:, h * 64:(h + 1) * 64])
                                yield tpf

            def outproj_items(b2, mcs):
                """Yield closures emitting one outproj matmul each (plus its
                eviction + DMA-out on vector/sync/gpsimd)."""
                for mc in mcs:
                    mrow = b2 * S + mc * 512
                    for et in range(D // 128):
                        def mm(et=et, mrow=mrow):
                            fp = ps8.tile([128, 512], f32, tag="mm", bufs=2,
                                          name="fp")
                            nc.tensor.matmul(fp[:],
                                             wo_sb[:, et * 128:(et + 1) * 128],
                                             outt_sb[:, mrow:mrow + 512],
                                             start=True, stop=True)
                            fo = fin.tile([128, 512], f32, tag="fo", name="fo")
                            nc.vector.tensor_copy(fo[:], fp[:])
                            eng = (nc.sync, nc.gpsimd)[et % 2]
                            eng.dma_start(
                                out_ap[et * 128:(et + 1) * 128,
                                       mrow:mrow + 512],
                                fo[:])
                        yield mm

            def attn_phase(b2, inject, budget):
                """Attention for batch b2. After each score pair, pull up to
                `budget` items from `inject` (other-phase PE work) so the
                tensor engine fills the scalar-exp-bound slots."""
                def pull(n):
                    for _ in range(n):
                        item = next(inject, None)
                        if item is None:
                            return
                        item()

                for qi in range(N_QCHUNK):
                    qcol = b2 * S + qi * 512
                    avp = [ps8.tile([128, 512], f32, tag=f"av{h}",
                                    name=f"av{h}")
                           for h in range(HEADS_PER_CORE)]
                    pending = None

                    def emit_av(kt, e):
                        first = (kt == 0)
                        last = (kt == N_KKTILE - 1)
                        for h in range(HEADS_PER_CORE):
                            nc.tensor.matmul(
                                avp[h][0:65, :],
                                v_sb[:, b2, kt, h, :],
                                e[:, h * 512:(h + 1) * 512],
                                start=first, stop=last)

                    for kt in range(N_KKTILE):
                        kkcol = b2 * S + kt * 128
                        sc = ps8.tile([128, 1024], f32, tag="sc", bufs=2,
                                      name="sc")
                        for h in range(HEADS_PER_CORE):
                            nc.tensor.matmul(
                                sc[:, h * 512:(h + 1) * 512],
                                qkb_sb[h * 64:(h + 1) * 64, kkcol:kkcol + 128],
                                qka_sb[h * 64:(h + 1) * 64, qcol:qcol + 512],
                                start=True, stop=True)
                        e = epool.tile([128, 1024], bf16, tag="e", bufs=4,
                                       name="e")
                        nc.scalar.activation(e[:], sc[:], Exp, scale=SCALE)
                        pull(budget)
                        if pending is not None:
                            emit_av(*pending)
                        pending = (kt, e)
                    emit_av(*pending)

                    for h in range(HEADS_PER_CORE):
                        st = stage.tile([128, 512], f32, tag="st", name="st")
                        nc.vector.tensor_copy(st[0:65, :], avp[h][0:65, :])
                        sidx = (b2 * N_QCHUNK + qi) * HEADS_PER_CORE + h
                        nc.gpsimd.dma_start(sums_dram[sidx:sidx + 1, :],
                                            st[64:65, :])
                        rb = stage.tile([128, 512], f32, tag="rb", name="rb")
                        nc.gpsimd.dma_start(
                            rb[0:64, :],
                            sums_dram[sidx:sidx + 1, :]
                            .partition_broadcast(64).squeeze(1))
                        rb2 = stage.tile([128, 512], f32, tag="rb2", name="rb2")
                        nc.vector.reciprocal_approx_fast(rb2[0:64, :],
                                                         rb[0:64, :])
                        if h == 0:
                            nc.vector.tensor_mul(outt_sb[0:64, qcol:qcol + 512],
                                                 st[0:64, :], rb2[0:64, :])
                        else:
                            tm = stage.tile([128, 512], bf16, tag="tm",
                                            name="tm")
                            nc.vector.tensor_mul(tm[0:64, :], st[0:64, :],
                                                 rb2[0:64, :])
                            nc.gpsimd.dma_start(
                                outt_sb[64:128, qcol:qcol + 512], tm[0:64, :])
                # emit whatever the slots didn't absorb before moving on
                pull(1 << 30)

            qkv_batch0()
            attn_phase(0, qkv_batch1_items(), budget=2)

            def b1_inject():
                yield from outproj_items(0, range(N_MCHUNK_B))
                yield from outproj_items(1, range(3))
            attn_phase(1, b1_inject(), budget=1)
            for item in outproj_items(1, [3]):
                item()
    nc.compile()
    return nc


def _shard_inputs(x, w_qkv, b_qkv, w_out):
    import ml_dtypes
    bf16 = ml_dtypes.bfloat16
    xt = np.ascontiguousarray(x.reshape(M, D).T.astype(bf16))  # (1024, 4096)
    ones = np.ones((128, 64), dtype=bf16)
    ident = np.eye(128, dtype=bf16)
    in_maps = []
    for c in range(N_CORES):
        h0 = HEADS_PER_CORE * c
        rows_q, rows_k, rows_v, dcols = [], [], [], []
        for h in (h0, h0 + 1):
            rows_q += list(range(h * 192, h * 192 + 64))
            rows_k += list(range(h * 192 + 64, h * 192 + 128))
            rows_v += list(range(h * 192 + 128, h * 192 + 192))
            dcols += list(range(h * 64, (h + 1) * 64))
        in_maps.append({
            "xt": xt,
            "wqa": np.ascontiguousarray(w_qkv[rows_q, :].T.astype(bf16)),
            "wqb": np.ascontiguousarray(w_qkv[rows_k, :].T.astype(bf16)),
            "wv": np.ascontiguousarray(w_qkv[rows_v, :].T.astype(bf16)),
            "wo": np.ascontiguousarray(w_out[:, dcols].T.astype(bf16)),
            "ba": np.ascontiguousarray(b_qkv[rows_q].reshape(128, 1)),
            "bb": np.ascontiguousarray(b_qkv[rows_k].reshape(128, 1)),
            "bv": np.ascontiguousarray(b_qkv[rows_v].reshape(128, 1)),
            "ones": ones,
            "ident": ident,
        })
    return in_maps


def kernel(x, w_qkv, b_qkv, w_out, b_out, _trace=False):
    from concourse.bass_utils import run_bass_kernel_spmd

    x = np.asarray(x, dtype=np.float32)
    w_qkv = np.asarray(w_qkv, dtype=np.float32)
    b_qkv = np.asarray(b_qkv, dtype=np.float32)
    w_out = np.asarray(w_out, dtype=np.float32)
    b_out = np.asarray(b_out, dtype=np.float32)

    if "nc" not in _CACHE:
        _CACHE["nc"] = _build_module()
    nc = _CACHE["nc"]

    in_maps = _shard_inputs(x, w_qkv, b_qkv, w_out)
    res = run_bass_kernel_spmd(nc, in_maps, list(range(N_CORES)), trace=_trace)
    acc = np.zeros((D, M), dtype=np.float64)
    for c in range(N_CORES):
        acc += res.results[c]["partial"]
    acc = acc.T + b_out
    out = acc.astype(np.float32).reshape(B, S, D)
    if _trace:
        _CACHE["last_exec_time_ns"] = res.exec_time_ns
        _CACHE["last_res"] = res
    return out


# revision 12
# speedup vs baseline: 1.3588x; 1.0769x over previous
"""Trainium2 Bass kernel for nn_MultiHeadAttention_83863531421896.

Full-input contract: kernel(**inputs) takes the unsharded tensors and
returns the full (2, 2048, 1024) output. Internally the 16 heads are
sharded 2-per-core across 8 NeuronCores (tensor parallel); each core
computes its heads' attention plus its slice of the output projection,
and the 8 partial projections are reduced on the host.

v3 design notes:
  - All matmul operands are bf16 (1 cycle/row on HW vs 2 for fp32),
    accumulation stays fp32 in PSUM. Host pre-casts x/weights; on-device
    casts are folded into existing PSUM->SBUF evictions for free.
  - exp runs on the Scalar engine in [128, 1024] tiles (both heads of a
    key-tile in one activation); the scalar queue carries nothing but
    the head x-DMAs (which finish before the first exp) and the exps.
  - The kernel head is minimal: K(b0), V(b0)+transposes, Q-chunk0(b0).
    Everything else (Q chunks 1-3 of b0, all of qkv(b1), both output
    projections) is injected instruction-by-instruction into the
    attention tensor-queue slots, so Tensor and Scalar run ~back to
    back for the whole span.
  - Partial output projection is written in bf16 (halves the out-DMA),
    summed across cores on the host in float64.
"""

import sys

if "/opt/trn_rl_repo" not in sys.path:
    sys.path.insert(0, "/opt/trn_rl_repo")

import numpy as np

B = 2
S = 2048
D = 1024
H = 16
HD = 64
N_CORES = 8
HEADS_PER_CORE = H // N_CORES  # 2
M = B * S                      # 4096 tokens
N_MCHUNK_B = S // 512          # 4 m-chunks of 512 tokens per batch
N_KTILE = D // 128             # 8 contraction tiles for qkv
N_QCHUNK = S // 512            # 4 q-chunks per batch
N_KKTILE = S // 128            # 16 key tiles per batch
SCALE = 1.0 / np.sqrt(HD)

_CACHE = {}


def _build_module():
    import concourse.bass as bass
    import concourse.tile as tile
    from concourse import bacc, mybir

    f32 = mybir.dt.float32
    bf16 = mybir.dt.bfloat16
    Exp = mybir.ActivationFunctionType.Exp

    nc = bacc.Bacc("TRN2", target_bir_lowering=False, debug=False,
                   num_devices=N_CORES)

    xt_ap = nc.dram_tensor("xt", [D, M], bf16, kind="ExternalInput").ap()
    wqa_ap = nc.dram_tensor("wqa", [D, 128], bf16, kind="ExternalInput").ap()
    wqb_ap = nc.dram_tensor("wqb", [D, 128], bf16, kind="ExternalInput").ap()
    wv_ap = nc.dram_tensor("wv", [D, 128], bf16, kind="ExternalInput").ap()
    wo_ap = nc.dram_tensor("wo", [128, D], bf16, kind="ExternalInput").ap()
    ba_ap = nc.dram_tensor("ba", [128, 1], f32, kind="ExternalInput").ap()
    bb_ap = nc.dram_tensor("bb", [128, 1], f32, kind="ExternalInput").ap()
    bv_ap = nc.dram_tensor("bv", [128, 1], f32, kind="ExternalInput").ap()
    ones_ap = nc.dram_tensor("ones", [128, 64], bf16, kind="ExternalInput").ap()
    ident_ap = nc.dram_tensor("ident", [128, 128], bf16,
                              kind="ExternalInput").ap()
    out_ap = nc.dram_tensor("partial", [D, M], bf16,
                            kind="ExternalOutput").ap()
    sums_dram = nc.dram_tensor(
        "sums_scratch", [B * N_QCHUNK * HEADS_PER_CORE, 512], f32).ap()

    with tile.TileContext(nc) as tc:
        with tc.tile_pool(name="persist", bufs=1) as persist, \
             tc.tile_pool(name="const", bufs=1) as const, \
             tc.tile_pool(name="xpool", bufs=1) as xpool, \
             tc.tile_pool(name="vt_pool", bufs=2) as vt_pool, \
             tc.tile_pool(name="ps8", bufs=1, space="PSUM") as ps8, \
             tc.tile_pool(name="epool", bufs=1) as epool, \
             tc.tile_pool(name="stage", bufs=2) as stage, \
             tc.tile_pool(name="fin", bufs=4) as fin:
            qka_sb = persist.tile([128, M], bf16, tag="qka")
            qkb_sb = persist.tile([128, M], bf16, tag="qkb")
            v_sb = persist.tile([128, B, N_KKTILE, HEADS_PER_CORE, 65], bf16,
                                tag="vsb")
            outt_sb = persist.tile([128, M], bf16, tag="outt")

            wo_sb = const.tile([128, D], bf16, tag="wo")
            nc.gpsimd.dma_start(wo_sb[:], wo_ap[:])
            ident_sb = const.tile([128, 128], bf16, tag="ident")
            nc.sync.dma_start(ident_sb[:], ident_ap[:])
            ba_sb = const.tile([128, 1], f32, tag="ba")
            nc.gpsimd.dma_start(ba_sb[:], ba_ap[:])
            bb_sb = const.tile([128, 1], f32, tag="bb")
            nc.sync.dma_start(bb_sb[:], bb_ap[:])
            bv_sb = const.tile([128, 1], f32, tag="bv")
            nc.gpsimd.dma_start(bv_sb[:], bv_ap[:])
            wq_sb = const.tile([128, 3, N_KTILE, 128], bf16, tag="wq")
            for ki in range(N_KTILE):
                eng = (nc.gpsimd, nc.sync, nc.scalar)[ki % 3]
                eng.dma_start(wq_sb[:, 0, ki], wqa_ap[ki * 128:(ki + 1) * 128, :])
                eng.dma_start(wq_sb[:, 1, ki], wqb_ap[ki * 128:(ki + 1) * 128, :])
                eng.dma_start(wq_sb[:, 2, ki], wv_ap[ki * 128:(ki + 1) * 128, :])
            nc.gpsimd.dma_start(
                v_sb[:, :, :, :, 64:65],
                ones_ap[:, 0:B * N_KKTILE * HEADS_PER_CORE].rearrange(
                    "p (b t h) -> p b t h", b=B, t=N_KKTILE)[:, :, :, :, None])

            # x staged fully; batch-0 tiles first (ki-major, matching the
            # K-group's consumption order) across all three DMA queues, then
            # batch-1 tiles on sync/gpsimd (scalar must be clean once the
            # exp stream starts).
            xs = xpool.tile([128, B * N_MCHUNK_B, N_KTILE, 512], bf16,
                            tag="xs")
            n = 0
            for ki in range(N_KTILE):
                for mc in range(N_MCHUNK_B):
                    eng = (nc.sync, nc.gpsimd, nc.scalar)[n % 3]
                    n += 1
                    eng.dma_start(
                        xs[:, mc, ki],
                        xt_ap[ki * 128:(ki + 1) * 128, mc * 512:(mc + 1) * 512])
            for mi in range(N_MCHUNK_B, 2 * N_MCHUNK_B):
                for ki in range(N_KTILE):
                    eng = (nc.sync, nc.gpsimd)[n % 2]
                    n += 1
                    eng.dma_start(
                        xs[:, mi, ki],
                        xt_ap[ki * 128:(ki + 1) * 128, mi * 512:(mi + 1) * 512])

            vts = [None, None]

            def qkv_group(b2, ei, mcs, psum_tags, vt_sb):
                """One projection group: ki-inner over the given m-chunks so
                each stationary weight tile is loaded once per 4 matmuls."""
                bias, dest, dcol = (
                    (ba_sb, qka_sb, b2 * S),
                    (bb_sb, qkb_sb, b2 * S),
                    (bv_sb, vt_sb, 0),
                )[ei]
                pss = []
                for mc, t in zip(mcs, psum_tags):
                    kw = {"bufs": 2} if t in ("sc", "mm") else {}
                    pss.append(ps8.tile([128, 512], f32, tag=t,
                                        name=f"qkvps{b2}{ei}{mc}", **kw))
                for ki in range(N_KTILE):
                    for j, mc in enumerate(mcs):
                        nc.tensor.matmul(pss[j][:], wq_sb[:, ei, ki],
                                         xs[:, b2 * N_MCHUNK_B + mc, ki],
                                         start=(ki == 0),
                                         stop=(ki == N_KTILE - 1))
                for j, mc in enumerate(mcs):
                    nc.vector.tensor_scalar_add(
                        dest[:, dcol + mc * 512:dcol + (mc + 1) * 512],
                        pss[j][:], bias[:])

            def transpose_v(b2, kt, vt_sb):
                tp = ps8.tile([128, 128], bf16, tag="mm", bufs=2,
                              name=f"tp{b2}")
                nc.tensor.transpose(tp[:], vt_sb[:, kt * 128:(kt + 1) * 128],
                                    ident_sb[:])
                for h in range(HEADS_PER_CORE):
                    nc.vector.tensor_copy(v_sb[:, b2, kt, h, 0:64],
                                          tp[:, h * 64:(h + 1) * 64])

            def head_b0():
                """Minimal pre-attention work: K(b0), V(b0)+transposes,
                Q-chunk0(b0)."""
                vt_sb = vt_pool.tile([128, S], bf16, tag="vt", name="vt0")
                vts[0] = vt_sb
                qkv_group(0, 1, range(4), ("sc", "sc", "av0", "av1"), vt_sb)
                qkv_group(0, 2, range(4), ("sc", "sc", "av0", "av1"), vt_sb)
                for kt in range(N_KKTILE):
                    transpose_v(0, kt, vt_sb)
                qkv_group(0, 0, [0], ("mm",), vt_sb)

            def q123_b0_items():
                for mc in range(1, N_MCHUNK_B):
                    ps = ps8.tile([128, 512], f32, tag="mm", bufs=2,
                                  name=f"q0ps{mc}")
                    for ki in range(N_KTILE):
                        def mm(ki=ki, ps=ps, mc=mc):
                            nc.tensor.matmul(ps[:], wq_sb[:, 0, ki],
                                             xs[:, mc, ki],
                                             start=(ki == 0),
                                             stop=(ki == N_KTILE - 1))
                        yield mm
                    def evict(ps=ps, mc=mc):
                        nc.vector.tensor_scalar_add(
                            qka_sb[:, mc * 512:(mc + 1) * 512], ps[:], ba_sb[:])
                    yield evict

            def qkv_b1_items():
                vt_sb = vt_pool.tile([128, S], bf16, tag="vt", name="vt1")
                vts[1] = vt_sb
                # v first so transposes spread, then k, then q
                for ei in (2, 1, 0):
                    bias, dest, dcol = (
                        (ba_sb, qka_sb, S),
                        (bb_sb, qkb_sb, S),
                        (bv_sb, vt_sb, 0),
                    )[ei]
                    for mc in range(N_MCHUNK_B):
                        mi = N_MCHUNK_B + mc
                        ps = ps8.tile([128, 512], f32, tag="mm", bufs=2,
                                      name=f"qkv1ps{ei}{mc}")
                        for ki in range(N_KTILE):
                            def mm(ki=ki, ps=ps, ei=ei, mi=mi):
                                nc.tensor.matmul(ps[:], wq_sb[:, ei, ki],
                                                 xs[:, mi, ki],
                                                 start=(ki == 0),
                                                 stop=(ki == N_KTILE - 1))
                            yield mm
                        def evict(ps=ps, dest=dest, bias=bias, dcol=dcol,
                                  mc=mc):
                            nc.vector.tensor_scalar_add(
                                dest[:, dcol + mc * 512:dcol + (mc + 1) * 512],
                                ps[:], bias[:])
                        yield evict
                        if ei == 2:
                            for kt in range(mc * 4, (mc + 1) * 4):
                                def tpf(kt=kt, vt_sb=vt_sb):
                                    transpose_v(1, kt, vt_sb)
                                yield tpf

            def outproj_items(b2, mcs):
                for mc in mcs:
                    mrow = b2 * S + mc * 512
                    for et in range(D // 128):
                        def mm(et=et, mrow=mrow):
                            fp = ps8.tile([128, 512], f32, tag="mm", bufs=2,
                                          name="fp")
                            nc.tensor.matmul(fp[:],
                                             wo_sb[:, et * 128:(et + 1) * 128],
                                             outt_sb[:, mrow:mrow + 512],
                                             start=True, stop=True)
                            fo = fin.tile([128, 512], bf16, tag="fo",
                                          name="fo")
                            nc.vector.tensor_copy(fo[:], fp[:])
                            eng = (nc.sync, nc.gpsimd)[et % 2]
                            eng.dma_start(
                                out_ap[et * 128:(et + 1) * 128,
                                       mrow:mrow + 512],
                                fo[:])
                        yield mm

            def attn_phase(b2, inject, budgets):
                """Attention for batch b2. After each score pair, pull up to
                budgets[qi] items from `inject` (other-phase PE work) so the
                tensor engine fills the scalar-exp-bound slots."""
                def pull(n):
                    for _ in range(n):
                        item = next(inject, None)
                        if item is None:
                            return
                        item()

                for qi in range(N_QCHUNK):
                    qcol = b2 * S + qi * 512
                    avp = [ps8.tile([128, 512], f32, tag=f"av{h}",
                                    name=f"av{h}")
                           for h in range(HEADS_PER_CORE)]
                    pending = None

                    def emit_av(kt, e):
                        first = (kt == 0)
                        last = (kt == N_KKTILE - 1)
                        for h in range(HEADS_PER_CORE):
                            nc.tensor.matmul(
                                avp[h][0:65, :],
                                v_sb[:, b2, kt, h, :],
                                e[:, h * 512:(h + 1) * 512],
                                start=first, stop=last)

                    for kt in range(N_KKTILE):
                        kkcol = b2 * S + kt * 128
                        sc = ps8.tile([128, 1024], f32, tag="sc", bufs=2,
                                      name="sc")
                        for h in range(HEADS_PER_CORE):
                            nc.tensor.matmul(
                                sc[:, h * 512:(h + 1) * 512],
                                qkb_sb[h * 64:(h + 1) * 64, kkcol:kkcol + 128],
                                qka_sb[h * 64:(h + 1) * 64, qcol:qcol + 512],
                                start=True, stop=True)
                        e = epool.tile([128, 1024], bf16, tag="e", bufs=4,
                                       name="e")
                        nc.scalar.activation(e[:], sc[:], Exp, scale=SCALE)
                        pull(budgets[qi])
                        if pending is not None:
                            emit_av(*pending)
                        pending = (kt, e)
                    emit_av(*pending)

                    for h in range(HEADS_PER_CORE):
                        st = stage.tile([128, 512], f32, tag="st", name="st")
                        nc.vector.tensor_copy(st[0:65, :], avp[h][0:65, :])
                        sidx = (b2 * N_QCHUNK + qi) * HEADS_PER_CORE + h
                        nc.gpsimd.dma_start(sums_dram[sidx:sidx + 1, :],
                                            st[64:65, :])
                        rb = stage.tile([128, 512], f32, tag="rb", name="rb")
                        nc.gpsimd.dma_start(
                            rb[0:64, :],
                            sums_dram[sidx:sidx + 1, :]
                            .partition_broadcast(64).squeeze(1))
                        rb2 = stage.tile([128, 512], f32, tag="rb2", name="rb2")
                        nc.vector.reciprocal_approx_fast(rb2[0:64, :],
                                                         rb[0:64, :])
                        if h == 0:
                            nc.vector.tensor_mul(outt_sb[0:64, qcol:qcol + 512],
                                                 st[0:64, :], rb2[0:64, :])
                        else:
                            tm = stage.tile([128, 512], bf16, tag="tm",
                                            name="tm")
                            nc.vector.tensor_mul(tm[0:64, :], st[0:64, :],
                                                 rb2[0:64, :])
                            nc.gpsimd.dma_start(
                                outt_sb[64:128, qcol:qcol + 512], tm[0:64, :])
                # emit whatever the slots didn't absorb before moving on
                pull(1 << 30)

            head_b0()

            def b0_inject():
                yield from q123_b0_items()
                yield from qkv_b1_items()
            attn_phase(0, b0_inject(), budgets=(2, 2, 2, 3))

            def b1_inject():
                yield from outproj_items(0, range(N_MCHUNK_B))
                yield from outproj_items(1, range(3))
            attn_phase(1, b1_inject(), budgets=(1, 1, 1, 1))
            for item in outproj_items(1, [3]):
                item()
    nc.compile()
    return nc


def _shard_inputs(x, w_qkv, b_qkv, w_out):
    import ml_dtypes
    bf16 = ml_dtypes.bfloat16
    xt = np.ascontiguousarray(x.reshape(M, D).T.astype(bf16))  # (1024, 4096)
    ones = np.ones((128, 64), dtype=bf16)
    ident = np.eye(128, dtype=bf16)
    in_maps = []
    for c in range(N_CORES):
        h0 = HEADS_PER_CORE * c
        rows_q, rows_k, rows_v, dcols = [], [], [], []
        for h in (h0, h0 + 1):
            rows_q += list(range(h * 192, h * 192 + 64))
            rows_k += list(range(h * 192 + 64, h * 192 + 128))
            rows_v += list(range(h * 192 + 128, h * 192 + 192))
            dcols += list(range(h * 64, (h + 1) * 64))
        in_maps.append({
            "xt": xt,
            "wqa": np.ascontiguousarray(w_qkv[rows_q, :].T.astype(bf16)),
            "wqb": np.ascontiguousarray(w_qkv[rows_k, :].T.astype(bf16)),
            "wv": np.ascontiguousarray(w_qkv[rows_v, :].T.astype(bf16)),
            "wo": np.ascontiguousarray(w_out[:, dcols].T.astype(bf16)),
            "ba": np.ascontiguousarray(b_qkv[rows_q].reshape(128, 1)),
            "bb": np.ascontiguousarray(b_qkv[rows_k].reshape(128, 1)),
            "bv": np.ascontiguousarray(b_qkv[rows_v].reshape(128, 1)),
            "ones": ones,
            "ident": ident,
        })
    return in_maps


def kernel(x, w_qkv, b_qkv, w_out, b_out, _trace=False):
    from concourse.bass_utils import run_bass_kernel_spmd

    x = np.asarray(x, dtype=np.float32)
    w_qkv = np.asarray(w_qkv, dtype=np.float32)
    b_qkv = np.asarray(b_qkv, dtype=np.float32)
    w_out = np.asarray(w_out, dtype=np.float32)
    b_out = np.asarray(b_out, dtype=np.float32)

    if "nc" not in _CACHE:
        _CACHE["nc"] = _build_module()
    nc = _CACHE["nc"]

    in_maps = _shard_inputs(x, w_qkv, b_qkv, w_out)
    res = run_bass_kernel_spmd(nc, in_maps, list(range(N_CORES)), trace=_trace)
    acc = np.zeros((D, M), dtype=np.float64)
    for c in range(N_CORES):
        acc += np.asarray(res.results[c]["partial"], dtype=np.float64)
    acc = acc.T + b_out
    out = acc.astype(np.float32).reshape(B, S, D)
    if _trace:
        _CACHE["last_exec_time_ns"] = res.exec_time_ns
        _CACHE["last_res"] = res
    return out


# revision 19
# speedup vs baseline: 1.4003x; 1.0306x over previous
"""Trainium2 Bass kernel for nn_MultiHeadAttention_83863531421896.

Full-input contract: kernel(**inputs) takes the unsharded tensors and
returns the full (2, 2048, 1024) output. Internally the 16 heads are
sharded 2-per-core across 8 NeuronCores (tensor parallel); each core
computes its heads' attention plus its slice of the output projection,
and the 8 partial projections are reduced on the host.

v4 design notes:
  - All matmul operands are bf16 (1 cycle/row on HW vs 2 for fp32),
    accumulation stays fp32 in PSUM. Host pre-casts x/weights; on-device
    casts are folded into existing PSUM->SBUF evictions for free.
  - exp runs on the Scalar engine in [128, 1024] tiles (both heads of a
    key-tile in one activation); the scalar queue carries only exps.
  - Minimal head: only K/V/Q-chunk0 of batch 0 (24 matmuls + 4
    transposes) run before attention; x DMA is ordered chunk-major so
    the first chunk lands ~3us in. Everything else (remaining K/V/Q
    chunks of b0, all of qkv(b1), both output projections) is injected
    instruction-by-instruction into attention's tensor-queue slots.
  - Softmax denominators: the AV matmul's ones-column gives per-query
    sums on PSUM partition 64; a 1-contraction-row matmul broadcasts
    that row to partitions 0..63 (no DRAM round-trip), then
    reciprocal_approx_fast + multiply normalize straight out of PSUM.
  - Partial output projection is written in bf16 (halves the out-DMA),
    summed across cores on the host in float64.
"""

import sys

if "/opt/trn_rl_repo" not in sys.path:
    sys.path.insert(0, "/opt/trn_rl_repo")

import numpy as np

B = 2
S = 2048
D = 1024
H = 16
HD = 64
N_CORES = 8
HEADS_PER_CORE = H // N_CORES  # 2
M = B * S                      # 4096 tokens
N_MCHUNK_B = S // 512          # 4 m-chunks of 512 tokens per batch
N_KTILE = D // 128             # 8 contraction tiles for qkv
N_QCHUNK = S // 512            # 4 q-chunks per batch
N_KKTILE = S // 128            # 16 key tiles per batch
SCALE = 1.0 / np.sqrt(HD)

_CACHE = {}


def _build_module():
    import concourse.bass as bass
    import concourse.tile as tile
    from concourse import bacc, mybir

    f32 = mybir.dt.float32
    bf16 = mybir.dt.bfloat16
    Exp = mybir.ActivationFunctionType.Exp

    nc = bacc.Bacc("TRN2", target_bir_lowering=False, debug=False,
                   num_devices=N_CORES)

    xt_ap = nc.dram_tensor("xt", [D, M], bf16, kind="ExternalInput").ap()
    wqa_ap = nc.dram_tensor("wqa", [D, 128], bf16, kind="ExternalInput").ap()
    wqb_ap = nc.dram_tensor("wqb", [D, 128], bf16, kind="ExternalInput").ap()
    wv_ap = nc.dram_tensor("wv", [D, 128], bf16, kind="ExternalInput").ap()
    wo_ap = nc.dram_tensor("wo", [128, D], bf16, kind="ExternalInput").ap()
    ba_ap = nc.dram_tensor("ba", [128, 1], f32, kind="ExternalInput").ap()
    bb_ap = nc.dram_tensor("bb", [128, 1], f32, kind="ExternalInput").ap()
    bv_ap = nc.dram_tensor("bv", [128, 1], f32, kind="ExternalInput").ap()
    ones_ap = nc.dram_tensor("ones", [128, 64], bf16, kind="ExternalInput").ap()
    ident_ap = nc.dram_tensor("ident", [128, 128], bf16,
                              kind="ExternalInput").ap()
    out_ap = nc.dram_tensor("partial", [D, M], bf16,
                            kind="ExternalOutput").ap()

    with tile.TileContext(nc) as tc:
        with tc.tile_pool(name="persist", bufs=1) as persist, \
             tc.tile_pool(name="const", bufs=1) as const, \
             tc.tile_pool(name="xpool", bufs=1) as xpool, \
             tc.tile_pool(name="vt_pool", bufs=2) as vt_pool, \
             tc.tile_pool(name="ps8", bufs=1, space="PSUM") as ps8, \
             tc.tile_pool(name="epool", bufs=1) as epool, \
             tc.tile_pool(name="stage", bufs=2) as stage, \
             tc.tile_pool(name="fin", bufs=4) as fin:
            qka_sb = persist.tile([128, M], bf16, tag="qka")
            qkb_sb = persist.tile([128, M], bf16, tag="qkb")
            v_sb = persist.tile([128, B, N_KKTILE, HEADS_PER_CORE, 65], bf16,
                                tag="vsb")
            outt_sb = persist.tile([128, M], bf16, tag="outt")

            # constants: weights first on every queue so the head can start
            wq_sb = const.tile([128, 3, N_KTILE, 128], bf16, tag="wq")
            for ki in range(N_KTILE):
                eng = (nc.gpsimd, nc.sync, nc.scalar)[ki % 3]
                eng.dma_start(wq_sb[:, 0, ki], wqa_ap[ki * 128:(ki + 1) * 128, :])
                eng.dma_start(wq_sb[:, 1, ki], wqb_ap[ki * 128:(ki + 1) * 128, :])
                eng.dma_start(wq_sb[:, 2, ki], wv_ap[ki * 128:(ki + 1) * 128, :])
            ident_sb = const.tile([128, 128], bf16, tag="ident")
            nc.scalar.dma_start(ident_sb[:], ident_ap[:])
            ba_sb = const.tile([128, 1], f32, tag="ba")
            nc.scalar.dma_start(ba_sb[:], ba_ap[:])
            bb_sb = const.tile([128, 1], f32, tag="bb")
            nc.scalar.dma_start(bb_sb[:], bb_ap[:])
            bv_sb = const.tile([128, 1], f32, tag="bv")
            nc.scalar.dma_start(bv_sb[:], bv_ap[:])
            ones_sb = const.tile([128, 64], bf16, tag="ones")
            nc.scalar.dma_start(ones_sb[:], ones_ap[:])
            wo_sb = const.tile([128, D], bf16, tag="wo")
            nc.gpsimd.dma_start(
                v_sb[:, :, :, :, 64:65],
                ones_ap[:, 0:B * N_KKTILE * HEADS_PER_CORE].rearrange(
                    "p (b t h) -> p b t h", b=B, t=N_KKTILE)[:, :, :, :, None])

            # x staged fully, chunk-major (all ki of m-chunk 0 first) so the
            # progressive head starts after ~1MB; sync+gpsimd queues only.
            xs = xpool.tile([128, B * N_MCHUNK_B, N_KTILE, 512], bf16,
                            tag="xs")
            n = 0
            for mi in range(B * N_MCHUNK_B):
                for ki in range(N_KTILE):
                    eng = (nc.sync, nc.gpsimd)[n % 2]
                    n += 1
                    eng.dma_start(
                        xs[:, mi, ki],
                        xt_ap[ki * 128:(ki + 1) * 128, mi * 512:(mi + 1) * 512])
            # wo isn't needed until the first out-projection (~100us in)
            nc.sync.dma_start(wo_sb[:], wo_ap[:])

            vts = [None, None]

            def qkv_chunk_items(b2, ei, mc, vt_sb):
                """One projection m-chunk: 8 accumulating matmuls + bias
                eviction (+ V transposes), one yielded closure each."""
                bias, dest, dcol = (
                    (ba_sb, qka_sb, b2 * S),
                    (bb_sb, qkb_sb, b2 * S),
                    (bv_sb, vt_sb, 0),
                )[ei]
                mi = b2 * N_MCHUNK_B + mc
                ps = ps8.tile([128, 512], f32, tag="mm", bufs=2,
                              name=f"qkvps{b2}{ei}{mc}")
                for ki in range(N_KTILE):
                    def mm(ki=ki, ps=ps, ei=ei, mi=mi):
                        nc.tensor.matmul(ps[:], wq_sb[:, ei, ki],
                                         xs[:, mi, ki],
                                         start=(ki == 0),
                                         stop=(ki == N_KTILE - 1))
                    yield mm
                def evict(ps=ps, dest=dest, bias=bias, dcol=dcol, mc=mc):
                    nc.vector.tensor_scalar_add(
                        dest[:, dcol + mc * 512:dcol + (mc + 1) * 512],
                        ps[:], bias[:])
                yield evict
                if ei == 2:
                    for kt in range(mc * 4, (mc + 1) * 4):
                        def tpf(kt=kt, vt_sb=vt_sb, b2=b2):
                            tp = ps8.tile([128, 128], bf16, tag="mm", bufs=2,
                                          name=f"tp{b2}")
                            nc.tensor.transpose(
                                tp[:], vt_sb[:, kt * 128:(kt + 1) * 128],
                                ident_sb[:])
                            for h in range(HEADS_PER_CORE):
                                nc.vector.tensor_copy(
                                    v_sb[:, b2, kt, h, 0:64],
                                    tp[:, h * 64:(h + 1) * 64])
                        yield tpf

            def head_b0():
                """Minimal pre-attention work: chunk 0 of K, V (+first 4
                transposes) and Q for batch 0."""
                vt_sb = vt_pool.tile([128, S], bf16, tag="vt", name="vt0")
                vts[0] = vt_sb
                for ei in (1, 2, 0):
                    for item in qkv_chunk_items(0, ei, 0, vt_sb):
                        item()

            def b0_items():
                # finish b0's K/V (chunk-interleaved: K2 before V1 would
                # starve AV; K first within each chunk pair keeps score
                # inputs ahead of the kt loop), then Q 1-3, then all of b1
                for mc in range(1, N_MCHUNK_B):
                    yield from qkv_chunk_items(0, 1, mc, vts[0])
                    yield from qkv_chunk_items(0, 2, mc, vts[0])
                for mc in range(1, N_MCHUNK_B):
                    yield from qkv_chunk_items(0, 0, mc, vts[0])
                vt_sb = vt_pool.tile([128, S], bf16, tag="vt", name="vt1")
                vts[1] = vt_sb
                for ei in (2, 1, 0):
                    for mc in range(N_MCHUNK_B):
                        yield from qkv_chunk_items(1, ei, mc, vt_sb)

            # emitted-item prerequisites for attn(b0): the kt loop must not
            # be EMITTED past injected producers it reads (emission order is
            # queue order; a read emitted before its writer is a race).
            # item ends: K1=9 V1=22 K2=31 V2=44 K3=53 V3=66 Q1=75 Q2=84 Q3=93
            _KEND = {0: 0, 1: 9, 2: 31, 3: 53}
            _VEND = {0: 0, 1: 22, 2: 44, 3: 66}
            _QEND = {0: 0, 1: 75, 2: 84, 3: 93}

            def b0_sc_prereq(qi, kt):
                return max(_KEND[kt // 4], _QEND[qi])

            def b0_av_prereq(qi, kt):
                return _VEND[kt // 4]

            def outproj_items(b2, mcs):
                for mc in mcs:
                    mrow = b2 * S + mc * 512
                    for et in range(D // 128):
                        def mm(et=et, mrow=mrow):
                            fp = ps8.tile([128, 512], f32, tag="mm", bufs=2,
                                          name="fp")
                            nc.tensor.matmul(fp[:],
                                             wo_sb[:, et * 128:(et + 1) * 128],
                                             outt_sb[:, mrow:mrow + 512],
                                             start=True, stop=True)
                            fo = fin.tile([128, 512], bf16, tag="fo",
                                          name="fo")
                            nc.vector.tensor_copy(fo[:], fp[:])
                            eng = (nc.sync, nc.gpsimd)[et % 2]
                            eng.dma_start(
                                out_ap[et * 128:(et + 1) * 128,
                                       mrow:mrow + 512],
                                fo[:])
                        yield mm

            def attn_phase(b2, inject, budgets, sc_prereq=None,
                           av_prereq=None):
                """Attention for batch b2. After each score pair, pull up to
                budgets[qi] items from `inject` (other-phase PE work) so the
                tensor engine fills the scalar-exp-bound slots. sc/av_prereq
                give the minimum injected-item count that must be emitted
                before the score pair / AV pair of a slot."""
                pulled = [0]

                def pull(n):
                    for _ in range(n):
                        item = next(inject, None)
                        if item is None:
                            return
                        pulled[0] += 1
                        item()

                def pull_to(n):
                    if n > pulled[0]:
                        pull(n - pulled[0])

                for qi in range(N_QCHUNK):
                    qcol = b2 * S + qi * 512
                    avp = [ps8.tile([128, 512], f32, tag=f"av{h}",
                                    name=f"av{h}")
                           for h in range(HEADS_PER_CORE)]
                    pending = None

                    def emit_av(kt, e):
                        first = (kt == 0)
                        last = (kt == N_KKTILE - 1)
                        for h in range(HEADS_PER_CORE):
                            nc.tensor.matmul(
                                avp[h][0:65, :],
                                v_sb[:, b2, kt, h, :],
                                e[:, h * 512:(h + 1) * 512],
                                start=first, stop=last)

                    for kt in range(N_KKTILE):
                        kkcol = b2 * S + kt * 128
                        if sc_prereq is not None:
                            pull_to(sc_prereq(qi, kt))
                        sc = ps8.tile([128, 1024], f32, tag="sc", bufs=2,
                                      name="sc")
                        for h in range(HEADS_PER_CORE):
                            nc.tensor.matmul(
                                sc[:, h * 512:(h + 1) * 512],
                                qkb_sb[h * 64:(h + 1) * 64, kkcol:kkcol + 128],
                                qka_sb[h * 64:(h + 1) * 64, qcol:qcol + 512],
                                start=True, stop=True)
                        e = epool.tile([128, 1024], bf16, tag="e", bufs=4,
                                       name="e")
                        nc.scalar.activation(e[:], sc[:], Exp, scale=SCALE)
                        pull(budgets[qi])
                        if pending is not None:
                            if av_prereq is not None:
                                pull_to(av_prereq(qi, pending[0]))
                            emit_av(*pending)
                        pending = (kt, e)
                    if av_prereq is not None:
                        pull_to(av_prereq(qi, pending[0]))
                    emit_av(*pending)

                    # normalization: broadcast the ones-column sums (PSUM
                    # partition 64) down to 0..63 with a 1-row matmul, then
                    # reciprocal + multiply straight out of PSUM.
                    for h in range(HEADS_PER_CORE):
                        stb = stage.tile([128, 512], bf16, tag="stb",
                                         name="stb")
                        nc.vector.tensor_copy(stb[64:65, :], avp[h][64:65, :])
                        rbp = ps8.tile([128, 512], f32, tag="mm", bufs=2,
                                       name="rbp")
                        nc.tensor.matmul(rbp[0:64, :], ones_sb[64:65, 0:64],
                                         stb[64:65, :], start=True, stop=True)
                        rb2 = stage.tile([128, 512], f32, tag="rb2",
                                         name="rb2")
                        nc.vector.reciprocal_approx_fast(rb2[0:64, :],
                                                         rbp[0:64, :])
                        if h == 0:
                            nc.vector.tensor_mul(outt_sb[0:64, qcol:qcol + 512],
                                                 avp[h][0:64, :], rb2[0:64, :])
                        else:
                            tm = stage.tile([128, 512], bf16, tag="tm",
                                            name="tm")
                            nc.vector.tensor_mul(tm[0:64, :], avp[h][0:64, :],
                                                 rb2[0:64, :])
                            nc.gpsimd.dma_start(
                                outt_sb[64:128, qcol:qcol + 512], tm[0:64, :])
                # emit whatever the slots didn't absorb before moving on
                pull(1 << 30)

            head_b0()
            attn_phase(0, b0_items(), budgets=(5, 4, 3, 2),
                       sc_prereq=b0_sc_prereq, av_prereq=b0_av_prereq)

            def b1_inject():
                yield from outproj_items(0, range(N_MCHUNK_B))
                yield from outproj_items(1, range(3))
            attn_phase(1, b1_inject(), budgets=(1, 1, 1, 1))
            for item in outproj_items(1, [3]):
                item()
    nc.compile()
    return nc


def _shard_inputs(x, w_qkv, b_qkv, w_out):
    import ml_dtypes
    bf16 = ml_dtypes.bfloat16
    xt = np.ascontiguousarray(x.reshape(M, D).T.astype(bf16))  # (1024, 4096)
    ones = np.ones((128, 64), dtype=bf16)
    ident = np.eye(128, dtype=bf16)
    in_maps = []
    for c in range(N_CORES):
        h0 = HEADS_PER_CORE * c
        rows_q, rows_k, rows_v, dcols = [], [], [], []
        for h in (h0, h0 + 1):
            rows_q += list(range(h * 192, h * 192 + 64))
            rows_k += list(range(h * 192 + 64, h * 192 + 128))
            rows_v += list(range(h * 192 + 128, h * 192 + 192))
            dcols += list(range(h * 64, (h + 1) * 64))
        in_maps.append({
            "xt": xt,
            "wqa": np.ascontiguousarray(w_qkv[rows_q, :].T.astype(bf16)),
            "wqb": np.ascontiguousarray(w_qkv[rows_k, :].T.astype(bf16)),
            "wv": np.ascontiguousarray(w_qkv[rows_v, :].T.astype(bf16)),
            "wo": np.ascontiguousarray(w_out[:, dcols].T.astype(bf16)),
            "ba": np.ascontiguousarray(b_qkv[rows_q].reshape(128, 1)),
            "bb": np.ascontiguousarray(b_qkv[rows_k].reshape(128, 1)),
            "bv": np.ascontiguousarray(b_qkv[rows_v].reshape(128, 1)),
            "ones": ones,
            "ident": ident,
        })
    return in_maps


def kernel(x, w_qkv, b_qkv, w_out, b_out, _trace=False):
    from concourse.bass_utils import run_bass_kernel_spmd

    x = np.asarray(x, dtype=np.float32)
    w_qkv = np.asarray(w_qkv, dtype=np.float32)
    b_qkv = np.asarray(b_qkv, dtype=np.float32)
    w_out = np.asarray(w_out, dtype=np.float32)
    b_out = np.asarray(b_out, dtype=np.float32)

    if "nc" not in _CACHE:
        _CACHE["nc"] = _build_module()
    nc = _CACHE["nc"]

    in_maps = _shard_inputs(x, w_qkv, b_qkv, w_out)
    res = run_bass_kernel_spmd(nc, in_maps, list(range(N_CORES)), trace=_trace)
    acc = np.zeros((D, M), dtype=np.float64)
    for c in range(N_CORES):
        acc += np.asarray(res.results[c]["partial"], dtype=np.float64)
    acc = acc.T + b_out
    out = acc.astype(np.float32).reshape(B, S, D)
    if _trace:
        _CACHE["last_exec_time_ns"] = res.exec_time_ns
        _CACHE["last_res"] = res
    return out
